# revision 7
# baseline (speedup 1.0000x reference)
"""Trainium2 Bass kernel for nn_Conv1d_NN (retrieval_knn).

Reference computation per batch b (B=32, C=16, N=2048, K=3, C_out=32):
  dist[n,m] = |x[:,n] - x[:,m]|^2        (N x N pairwise distances)
  idx[n,:]  = argmin-3 of dist[n,:]      (self included)
  out[o,n]  = sum_{c,k} W[o,c,k] * x[c, idx[n,k]] + bias[o]

Kernel strategy (data-parallel over batch, 4 batches / core x 8 cores):
  * top-3 neighbours of row n are the top-3 LARGEST of
      score[n,m] = 2*gram[n,m] - sq[m]           (sq[n] row-const dropped)
    computed on-chip as ONE matmul per 128-row tile.  The matmul runs in
    bf16 hi/lo SPLIT precision (1 PE cycle/row instead of fp32's 4):
      x = xh + xl (bf16 halves), sq = sqh + sql + sqll (bf16 thirds)
      lhsT = [1;1;1; 2xh; 2xl; 2xh; 2xl]   (67 x 128 cols per tile)
      rhs  = [-sqh;-sql;-sqll; xh; xh; xl; xl]
    contraction 67 rounds up to the full-array 128 LDW path; all cross
    terms (hh, lh, hl, ll) are kept so the score error is ~1e-4 --
    top-3 selection flips only a handful of near-tied rows (rel err
    ~2-4e-3, vs 8e-4 for exact fp32, tolerance 2e-2).
  * DVE max/max_index (top-8 streaming sort) give values+indices per row.
    These two full passes (2 x 2048 cycles/tile) are the structural
    bottleneck: no TRN2 instruction recovers per-row top-k indices in
    fewer DVE cycles (Max/MaxIndex support no 2x/4x perf modes, and
    per-partition-dynamic gathers don't exist off gpsimd's 16-shared
    wrapped layout).
  * Score tile t covers rows n = 16*r + t so that a single DMA-transpose
    of the slot-major index tile lands EXACTLY in the int16 "wrapped"
    layout that gpsimd.ap_gather expects, with gather column j == n.
  * gather builds prime[(k,c), n] = x[c, idx[n,k]]; a zero-padded K=128
    matmul with W[(k,c), o] contracts gather+conv in one shot (fp32r:
    1 PE cycle/row, ~1e-4 relative noise -- output-domain only); bias is
    applied on the PSUM->SBUF move (ACT Identity with per-partition bias).

Schedule notes (DVE-bound; everything else hides under the ~302us of
Max/MaxIndex):
  * each batch's tail (index transpose / gather / conv / store) is emitted
    AFTER the next batch's score tiles to avoid PE head-of-line blocking;
  * the DMA transpose lives on the Activation DGE queue so its xbar-mode
    switch cannot serialize the SP load queue;
  * L/R are double-buffered by batch parity and prefetched one batch ahead;
    gather sources are prefetched one batch ahead on the scalar queue;
  * x^2 squaring runs on ACT (Square activation), not DVE;
  * the startup chain (x -> x^2 -> -sum x^2 -> sq splits -> R/L -> first
    score matmul) is cut into 512-column chunks so batch 0's first DVE
    Max gates on ~1/4 of the head instead of half of it.
"""

import numpy as np

import concourse.bass as bass
import concourse.bacc as bacc
import concourse.mybir as mybir
from concourse import tile
from concourse.bass_utils import run_bass_kernel_spmd

F32 = mybir.dt.float32
F32R = mybir.dt.float32r
BF16 = mybir.dt.bfloat16
U16 = mybir.dt.uint16
I16 = mybir.dt.int16
AF = mybir.ActivationFunctionType

NCORES = 8
B, C, N, K, CO = 32, 16, 2048, 3, 32
NB = B // NCORES          # batches per core
NT = N // 128             # score tiles per batch
P = 128                   # padded contraction size (conv matmul)
CK = 3 + 4 * C            # score contraction: 3 sq rows + [2xh;2xl;2xh;2xl]


def build_kernel() -> bass.Bass:
    nc = bacc.Bacc("TRN2", target_bir_lowering=False, debug=False)
    # lh[b] = [1;1;1; 2xh; 2xl; 2xh; 2xl]   (bf16 lhsT rows 0..66)
    lh = nc.dram_tensor("lh", [NB, CK, N], BF16, kind="ExternalInput")
    # xr16[b] = [xh; xh; xl; xl]            (bf16 rhs rows 3..66)
    xr16 = nc.dram_tensor("xr16", [NB, 4 * C, N], BF16, kind="ExternalInput")
    # xrep[b] = [x_b ; x_b ; x_b]           (gather source, f32)
    xrep = nc.dram_tensor("xrep", [NB, K * C, N], F32, kind="ExternalInput")
    # wg[(k*16+c), o] = W[o, c, k], zero-padded to 128 rows
    wg = nc.dram_tensor("wg", [P, CO], F32, kind="ExternalInput")
    bias = nc.dram_tensor("bias", [CO, 1], F32, kind="ExternalInput")
    # oblk = NEGATED block-diagonal ones, zero-padded to 128 rows
    oblk = nc.dram_tensor("oblk", [P, NB], F32, kind="ExternalInput")
    y = nc.dram_tensor("y", [NB, CO, N], F32, kind="ExternalOutput")

    with tile.TileContext(nc) as tc:
        with (
            tc.tile_pool(name="const", bufs=1) as cpool,
            tc.tile_pool(name="work", bufs=2) as wpool,
            tc.tile_pool(name="padded", bufs=1) as zpool,
            tc.tile_pool(name="smat", bufs=4) as spool,
            tc.tile_pool(name="small", bufs=4) as mpool,
            tc.tile_pool(name="psum", bufs=4, space="PSUM") as ppool,
        ):
            wg_sb = cpool.tile([P, CO], F32)
            nc.scalar.dma_start(wg_sb[:], wg[:])
            bias_sb = cpool.tile([CO, 1], F32)
            nc.scalar.dma_start(bias_sb[:], bias[:])
            oblk_sb = cpool.tile([P, NB], F32)
            nc.scalar.dma_start(oblk_sb[:], oblk[:])

            # xsq zero-padded to 128 rows for the oblk contraction.
            xsq = zpool.tile([P, N], F32, tag="xsq")
            nc.gpsimd.memset(xsq[:], 0.0)
            # score operands: bf16, contraction CK=67 (no padding needed --
            # 67 > 64 rounds the PE tile up to the full 128 LDW path).
            Ls, Rs = [], []
            for i in range(2):
                Ls.append(zpool.tile([CK, N], BF16, tag=f"L{i}",
                                     name=f"L{i}"))
                Rs.append(zpool.tile([CK, N], BF16, tag=f"R{i}",
                                     name=f"R{i}"))
            # conv gather target, zero rows 48..127 for the K=128 matmul
            pr = zpool.tile([P, N], F32, tag="pr")
            nc.gpsimd.memset(pr[:], 0.0)
            # gather-source tiles; loads staggered across batches (scalar DGE)
            xrs = [zpool.tile([K * C, N], F32, tag=f"xr{b}", name=f"xr{b}")
                   for b in range(NB)]
            nc.scalar.dma_start(xrs[0][:], xrep[0])

            # head chain in 512-column chunks: x -> x^2 (ACT) -> -sum x^2
            # (PE, exact fp32) -> sq hi/lo/lolo splits (ACT + DVE).
            x4 = cpool.tile([NB * C, N], F32)
            nsq_sb = cpool.tile([NB, N], F32)
            sqh = cpool.tile([NB, N], BF16)
            sql = cpool.tile([NB, N], BF16)
            sqll = cpool.tile([NB, N], BF16)
            r1f = cpool.tile([NB, N], F32)
            for ch in range(4):
                sl = slice(ch * 512, (ch + 1) * 512)
                q = nc.sync if ch % 2 == 0 else nc.scalar
                q.dma_start(x4[:, sl], xrep[:, 0:C, sl])
                nc.scalar.activation(xsq[0:NB * C, sl], x4[:, sl], AF.Square)
                psq = ppool.tile([NB, 512], F32, tag="ps")
                nc.tensor.matmul(psq[:], oblk_sb[:], xsq[:, sl],
                                 start=True, stop=True)
                nc.scalar.copy(nsq_sb[:, sl], psq[:])
                # sq = sqh + sql + sqll (bf16 thirds; nsq is already -sq)
                nc.scalar.activation(sqh[:, sl], nsq_sb[:, sl], AF.Copy)
                nc.gpsimd.tensor_sub(r1f[:, sl], nsq_sb[:, sl], sqh[:, sl])
                nc.scalar.activation(sql[:, sl], r1f[:, sl], AF.Copy)
                nc.gpsimd.tensor_sub(sqll[:, sl], r1f[:, sl], sql[:, sl])

            def load_LR(b, chunks=1):
                # R = [-sqh;-sql;-sqll; xh; xh; xl; xl], L from host.
                R = Rs[b % 2]
                L = Ls[b % 2]
                w = N // chunks
                for c in range(chunks):
                    sl = slice(c * w, (c + 1) * w)
                    nc.sync.dma_start(R[0:1, sl], sqh[b:b + 1, sl])
                    nc.sync.dma_start(R[1:2, sl], sql[b:b + 1, sl])
                    nc.sync.dma_start(R[2:3, sl], sqll[b:b + 1, sl])
                    nc.sync.dma_start(R[3:CK, sl], xr16[b][:, sl])
                    nc.scalar.dma_start(L[0:CK, sl], lh[b][:, sl])

            def tail(b, idxw):
                # one DMA transpose -> ap_gather wrapped int16 layout
                # (on the Activation DGE queue: the xbar-mode switch
                #  serializes the queue, so keep it off the SP load queue)
                TT = wpool.tile([128, 128], I16, tag="TT")
                nc.scalar.dma_start(TT[:], idxw[:].bitcast(I16),
                                    transpose=True)
                # gather neighbour columns: pr[(k,c), n] = x[c, idx[n,k]]
                nc.gpsimd.ap_gather(pr[0:K * C, :], xrs[b][:], TT[0:K * C, :],
                                    channels=K * C, num_elems=N, d=1,
                                    num_idxs=N)
                # conv == contraction over (k,c); bias on the PSUM->SBUF move
                ob = wpool.tile([CO, N], F32, tag="ob")
                for h in range(2):
                    po = ppool.tile([CO, N // 2], F32, tag="ps")
                    for ch in range(2):
                        sl = slice(ch * 512, (ch + 1) * 512)
                        gl = slice(h * 1024 + ch * 512,
                                   h * 1024 + (ch + 1) * 512)
                        nc.tensor.matmul(po[:, sl], wg_sb[:], pr[:, gl],
                                         start=True, stop=True)
                    hl = slice(h * 1024, (h + 1) * 1024)
                    nc.scalar.activation(ob[:, hl], po[:], AF.Identity,
                                         bias=bias_sb[:])
                    nc.sync.dma_start(y[b, :, hl], ob[:, hl])

            load_LR(0, chunks=4)
            pending = None
            for b in range(NB):
                R = Rs[b % 2]
                L = Ls[b % 2]

                # slot-major top-8 index tile: idxw[r, 16*slot + t]
                idxw = wpool.tile([128, 128], U16, tag="idxw")
                idxwv = idxw[:].rearrange("p (s g) -> p g s", g=16)
                Lv = L[:].rearrange("p (r g) -> p g r", g=16)

                for t in range(NT):
                    S = spool.tile([128, N], F32, tag="S")
                    for h in range(2):
                        ps = ppool.tile([128, N // 2], F32, tag="ps")
                        for ch in range(2):
                            sl = slice(ch * 512, (ch + 1) * 512)
                            gl = slice(h * 1024 + ch * 512,
                                       h * 1024 + (ch + 1) * 512)
                            nc.tensor.matmul(ps[:, sl], Lv[:, t, :], R[:, gl],
                                             start=True, stop=True)
                        nc.scalar.copy(S[:, h * 1024:(h + 1) * 1024], ps[:])
                    mx = mpool.tile([128, 8], F32, tag="mx")
                    nc.vector.max(mx[:], S[:])
                    nc.vector.max_index(idxwv[:, t, :], mx[:], S[:])
                    if t == 0 and b + 1 < NB:
                        load_LR(b + 1)   # prefetch next batch's operands
                    if t == 1 and b + 1 < NB:
                        nc.scalar.dma_start(xrs[b + 1][:], xrep[b + 1])

                # defer this batch's tail past the next batch's score tiles
                # to avoid PE head-of-line blocking on the gather chain
                if pending is not None:
                    tail(*pending)
                pending = (b, idxw)
            tail(*pending)
    nc.finalize()
    return nc


_CACHED_NC = None


def _get_nc():
    global _CACHED_NC
    if _CACHED_NC is None:
        _CACHED_NC = build_kernel()
    return _CACHED_NC


def _bf16(a: np.ndarray) -> np.ndarray:
    """Round f32 -> bf16 (round-to-nearest-even), returned as float32."""
    u = a.astype(np.float32).view(np.uint32)
    rounded = (u + 0x7FFF + ((u >> 16) & 1)) & 0xFFFF0000
    return rounded.view(np.float32)


def run(x, W, b, trace=False):
    x = np.asarray(x, dtype=np.float32)
    W = np.asarray(W, dtype=np.float32)
    b = np.asarray(b, dtype=np.float32)
    # wg[(k*16+c), o] = W[o, c, k], zero-padded to 128 rows
    wg = np.zeros((P, CO), np.float32)
    wg[:K * C] = W.transpose(2, 1, 0).reshape(K * C, CO)
    bias = np.ascontiguousarray(b.reshape(CO, 1))
    oblk = np.zeros((P, NB), np.float32)
    oblk[:NB * C] = -np.kron(np.eye(NB, dtype=np.float32),
                             np.ones((C, 1), np.float32))

    import ml_dtypes
    ones_plane = np.ones((NB, 3, N), np.float32)

    nc = _get_nc()
    in_maps = []
    for i in range(NCORES):
        xs = x[NB * i:NB * (i + 1)]                      # (NB, C, N) f32
        xh = _bf16(xs)
        xl = _bf16(xs - xh)
        # lhsT rows: [1,1,1, 2xh(16), 2xl(16), 2xh(16), 2xl(16)]
        lhf = np.concatenate(
            [ones_plane, 2 * xh, 2 * xl, 2 * xh, 2 * xl], axis=1)
        lhb = lhf.astype(ml_dtypes.bfloat16)
        # rhs x-rows: [xh, xh, xl, xl]
        xrf = np.concatenate([xh, xh, xl, xl], axis=1)
        xrb = xrf.astype(ml_dtypes.bfloat16)
        xrv = np.ascontiguousarray(np.concatenate([xs, xs, xs], axis=1))
        in_maps.append({"lh": np.ascontiguousarray(lhb),
                        "xr16": np.ascontiguousarray(xrb),
                        "xrep": xrv,
                        "wg": wg, "bias": bias, "oblk": oblk})
    res = run_bass_kernel_spmd(nc, in_maps, core_ids=list(range(NCORES)),
                               trace=trace)
    return np.concatenate([r["y"] for r in res.results], axis=0), res


def kernel(x: np.ndarray, W: np.ndarray, b: np.ndarray, **kw) -> np.ndarray:
    return run(x, W, b)[0]


# revision 15
# speedup vs baseline: 1.0066x; 1.0066x over previous
"""Trainium2 Bass kernel for nn_Conv1d_NN (retrieval_knn).

Reference computation per batch b (B=32, C=16, N=2048, K=3, C_out=32):
  dist[n,m] = |x[:,n] - x[:,m]|^2        (N x N pairwise distances)
  idx[n,:]  = argmin-3 of dist[n,:]      (self included)
  out[o,n]  = sum_{c,k} W[o,c,k] * x[c, idx[n,k]] + bias[o]

Kernel strategy (data-parallel over batch, 4 batches / core x 8 cores):
  * top-3 neighbours of row n are the top-3 LARGEST of
      score[n,m] = 2*gram[n,m] - sq[m]           (sq[n] row-const dropped)
    computed on-chip as ONE matmul per 128-row tile.  The matmul runs in
    bf16 hi/lo SPLIT precision (1 PE cycle/row instead of fp32's 4):
      x = xh + xl (bf16 halves), sq = sqh + sql + sqll (bf16 thirds)
      lhsT = [1;1;1; 2xh; 2xl; 2xh; 2xl]   (67 x 128 cols per tile)
      rhs  = [-sqh;-sql;-sqll; xh; xh; xl; xl]
    contraction 67 rounds up to the full-array 128 LDW path; all cross
    terms (hh, lh, hl, ll) are kept so the score error is ~1e-4 --
    top-3 selection flips only a handful of near-tied rows (rel err
    ~2-4e-3, vs 8e-4 for exact fp32, tolerance 2e-2).
  * DVE max/max_index (top-8 streaming sort) give values+indices per row.
    These two full passes (2 x 2048 cycles/tile) are the structural
    bottleneck: no TRN2 instruction recovers per-row top-k indices in
    fewer DVE cycles (Max/MaxIndex support no 2x/4x perf modes, and
    per-partition-dynamic gathers don't exist off gpsimd's 16-shared
    wrapped layout).
  * Score tile t covers rows n = 16*r + t so that a single DMA-transpose
    of the slot-major index tile lands EXACTLY in the int16 "wrapped"
    layout that gpsimd.ap_gather expects, with gather column j == n.
  * gather builds prime[(k,c), n] = x[c, idx[n,k]]; a zero-padded K=128
    matmul with W[(k,c), o] contracts gather+conv in one shot (fp32r:
    1 PE cycle/row, ~1e-4 relative noise -- output-domain only); bias is
    applied on the PSUM->SBUF move (ACT Identity with per-partition bias).

Schedule notes (DVE-bound; everything else hides under the ~302us of
Max/MaxIndex):
  * each batch's tail (index transpose / gather / conv / store) is emitted
    AFTER the next batch's score tiles to avoid PE head-of-line blocking;
  * the DMA transpose lives on the Activation DGE queue so its xbar-mode
    switch cannot serialize the SP load queue;
  * L/R are double-buffered by batch parity and prefetched one batch ahead;
    gather sources are prefetched one batch ahead on the scalar queue;
  * x^2 squaring runs on ACT (Square activation), not DVE;
  * the startup chain (x -> x^2 -> -sum x^2 -> sq splits -> R/L -> first
    score matmul) is cut into 512-column chunks so batch 0's first DVE
    Max gates on ~1/4 of the head instead of half of it.
"""

import numpy as np

import concourse.bass as bass
import concourse.bacc as bacc
import concourse.mybir as mybir
from concourse import tile
from concourse.bass_utils import run_bass_kernel_spmd

F32 = mybir.dt.float32
F32R = mybir.dt.float32r
BF16 = mybir.dt.bfloat16
U16 = mybir.dt.uint16
I16 = mybir.dt.int16
AF = mybir.ActivationFunctionType

NCORES = 8
B, C, N, K, CO = 32, 16, 2048, 3, 32
NB = B // NCORES          # batches per core
NT = N // 128             # score tiles per batch
P = 128                   # padded contraction size (conv matmul)
CK = 3 + 4 * C            # score contraction: 3 sq rows + [2xh;2xl;2xh;2xl]


def build_kernel() -> bass.Bass:
    nc = bacc.Bacc("TRN2", target_bir_lowering=False, debug=False)
    # lh[b] = [1;1;1; 2xh; 2xl; 2xh; 2xl]   (bf16 lhsT rows 0..66)
    lh = nc.dram_tensor("lh", [NB, CK, N], BF16, kind="ExternalInput")
    # xr16[b] = [xh; xh; xl; xl]            (bf16 rhs rows 3..66)
    xr16 = nc.dram_tensor("xr16", [NB, 4 * C, N], BF16, kind="ExternalInput")
    # xrep[b] = [x_b ; x_b ; x_b]           (gather source, f32)
    xrep = nc.dram_tensor("xrep", [NB, K * C, N], F32, kind="ExternalInput")
    # wg[(k*16+c), o] = W[o, c, k], zero-padded to 128 rows
    wg = nc.dram_tensor("wg", [P, CO], F32, kind="ExternalInput")
    bias = nc.dram_tensor("bias", [CO, 1], F32, kind="ExternalInput")
    # oblk = NEGATED block-diagonal ones, zero-padded to 128 rows
    oblk = nc.dram_tensor("oblk", [P, NB], F32, kind="ExternalInput")
    y = nc.dram_tensor("y", [NB, CO, N], F32, kind="ExternalOutput")

    with tile.TileContext(nc) as tc:
        with (
            tc.tile_pool(name="const", bufs=1) as cpool,
            tc.tile_pool(name="work", bufs=2) as wpool,
            tc.tile_pool(name="padded", bufs=1) as zpool,
            tc.tile_pool(name="smat", bufs=4) as spool,
            tc.tile_pool(name="small", bufs=4) as mpool,
            tc.tile_pool(name="psum", bufs=4, space="PSUM") as ppool,
        ):
            wg_sb = cpool.tile([P, CO], F32)
            nc.scalar.dma_start(wg_sb[:], wg[:])
            bias_sb = cpool.tile([CO, 1], F32)
            nc.scalar.dma_start(bias_sb[:], bias[:])
            oblk_sb = cpool.tile([P, NB], F32)
            nc.scalar.dma_start(oblk_sb[:], oblk[:])

            # xsq zero-padded to 128 rows for the oblk contraction.
            xsq = zpool.tile([P, N], F32, tag="xsq")
            nc.gpsimd.memset(xsq[:], 0.0)
            # score operands: bf16, contraction CK=67 (no padding needed --
            # 67 > 64 rounds the PE tile up to the full 128 LDW path).
            Ls, Rs = [], []
            for i in range(2):
                Ls.append(zpool.tile([CK, N], BF16, tag=f"L{i}",
                                     name=f"L{i}"))
                Rs.append(zpool.tile([CK, N], BF16, tag=f"R{i}",
                                     name=f"R{i}"))
            # conv gather target, zero rows 48..127 for the K=128 matmul
            pr = zpool.tile([P, N], F32, tag="pr")
            nc.gpsimd.memset(pr[:], 0.0)
            # gather-source tiles; loads staggered across batches (scalar DGE)
            xrs = [zpool.tile([K * C, N], F32, tag=f"xr{b}", name=f"xr{b}")
                   for b in range(NB)]
            nc.scalar.dma_start(xrs[0][:], xrep[0])

            # head chain in 512-column chunks: x -> x^2 (ACT) -> -sum x^2
            # (PE, exact fp32) -> sq hi/lo/lolo splits (ACT + DVE).
            x4 = cpool.tile([NB * C, N], F32)
            nsq_sb = cpool.tile([NB, N], F32)
            # sq3[b, j*N + n] = j-th bf16 third of batch b's -sq; engine
            # writes stay lane-aligned (partitions 0..3), the R-load DMA
            # unfolds the j column-blocks onto R partitions 0..2.
            sq3 = cpool.tile([NB, 3 * N], BF16)
            sq3v = sq3[:].rearrange("b (j n) -> b j n", j=3)
            r1f = cpool.tile([NB, N], F32)
            for ch in range(4):
                sl = slice(ch * 512, (ch + 1) * 512)
                nc.sync.dma_start(x4[:, sl], xrep[:, 0:C, sl])
                nc.scalar.activation(xsq[0:NB * C, sl], x4[:, sl], AF.Square)
                psq = ppool.tile([NB, 512], F32, tag="ps")
                nc.tensor.matmul(psq[:], oblk_sb[:], xsq[:, sl],
                                 start=True, stop=True)
                nc.scalar.copy(nsq_sb[:, sl], psq[:])
                # sq = three bf16 thirds (nsq is already -sq)
                nc.scalar.activation(sq3v[:, 0, sl], nsq_sb[:, sl], AF.Copy)
                nc.gpsimd.tensor_sub(r1f[:, sl], nsq_sb[:, sl],
                                     sq3v[:, 0, sl])
                nc.scalar.activation(sq3v[:, 1, sl], r1f[:, sl], AF.Copy)
                nc.gpsimd.tensor_sub(sq3v[:, 2, sl], r1f[:, sl],
                                     sq3v[:, 1, sl])

            def load_LR(b, chunks=1):
                # R = [-sqh;-sql;-sqll; xh; xh; xl; xl], L from host.
                R = Rs[b % 2]
                L = Ls[b % 2]
                w = N // chunks
                for c in range(chunks):
                    sl = slice(c * w, (c + 1) * w)
                    nc.sync.dma_start(
                        R[0:3, sl],
                        sq3[b:b + 1, :].rearrange(
                            "b (j n) -> b j n", j=3)[:, :, sl])
                    nc.sync.dma_start(R[3:CK, sl], xr16[b][:, sl])
                    nc.sync.dma_start(L[0:CK, sl], lh[b][:, sl])

            def tail(b, idxw):
                # one DMA transpose -> ap_gather wrapped int16 layout
                # (on the Activation DGE queue: the xbar-mode switch
                #  serializes the queue, so keep it off the SP load queue)
                TT = wpool.tile([128, 128], I16, tag="TT")
                nc.scalar.dma_start(TT[:], idxw[:].bitcast(I16),
                                    transpose=True)
                # gather neighbour columns: pr[(k,c), n] = x[c, idx[n,k]]
                nc.gpsimd.ap_gather(pr[0:K * C, :], xrs[b][:], TT[0:K * C, :],
                                    channels=K * C, num_elems=N, d=1,
                                    num_idxs=N)
                # conv == contraction over (k,c); bias on the PSUM->SBUF move
                ob = wpool.tile([CO, N], F32, tag="ob")
                for h in range(2):
                    po = ppool.tile([CO, N // 2], F32, tag="ps")
                    for ch in range(2):
                        sl = slice(ch * 512, (ch + 1) * 512)
                        gl = slice(h * 1024 + ch * 512,
                                   h * 1024 + (ch + 1) * 512)
                        nc.tensor.matmul(po[:, sl], wg_sb[:], pr[:, gl],
                                         start=True, stop=True)
                    hl = slice(h * 1024, (h + 1) * 1024)
                    nc.scalar.activation(ob[:, hl], po[:], AF.Identity,
                                         bias=bias_sb[:])
                    nc.sync.dma_start(y[b, :, hl], ob[:, hl])

            load_LR(0, chunks=4)
            pending = None
            for b in range(NB):
                R = Rs[b % 2]
                L = Ls[b % 2]

                # slot-major top-8 index tile: idxw[r, 16*slot + t]
                idxw = wpool.tile([128, 128], U16, tag="idxw")
                idxwv = idxw[:].rearrange("p (s g) -> p g s", g=16)
                Lv = L[:].rearrange("p (r g) -> p g r", g=16)

                for t in range(NT):
                    S = spool.tile([128, N], F32, tag="S")
                    for h in range(2):
                        ps = ppool.tile([128, N // 2], F32, tag="ps")
                        for ch in range(2):
                            sl = slice(ch * 512, (ch + 1) * 512)
                            gl = slice(h * 1024 + ch * 512,
                                       h * 1024 + (ch + 1) * 512)
                            nc.tensor.matmul(ps[:, sl], Lv[:, t, :], R[:, gl],
                                             start=True, stop=True)
                        nc.scalar.copy(S[:, h * 1024:(h + 1) * 1024], ps[:])
                    mx = mpool.tile([128, 8], F32, tag="mx")
                    nc.vector.max(mx[:], S[:])
                    nc.vector.max_index(idxwv[:, t, :], mx[:], S[:])
                    if t == 0 and b + 1 < NB:
                        load_LR(b + 1)   # prefetch next batch's operands
                    if t == 1 and b + 1 < NB:
                        nc.scalar.dma_start(xrs[b + 1][:], xrep[b + 1])

                # defer this batch's tail past the next batch's score tiles
                # to avoid PE head-of-line blocking on the gather chain
                if pending is not None:
                    tail(*pending)
                pending = (b, idxw)
            tail(*pending)
    nc.finalize()
    return nc


_CACHED_NC = None


def _get_nc():
    global _CACHED_NC
    if _CACHED_NC is None:
        _CACHED_NC = build_kernel()
    return _CACHED_NC


def _bf16(a: np.ndarray) -> np.ndarray:
    """Round f32 -> bf16 (round-to-nearest-even), returned as float32."""
    u = a.astype(np.float32).view(np.uint32)
    rounded = (u + 0x7FFF + ((u >> 16) & 1)) & 0xFFFF0000
    return rounded.view(np.float32)


def run(x, W, b, trace=False):
    x = np.asarray(x, dtype=np.float32)
    W = np.asarray(W, dtype=np.float32)
    b = np.asarray(b, dtype=np.float32)
    # wg[(k*16+c), o] = W[o, c, k], zero-padded to 128 rows
    wg = np.zeros((P, CO), np.float32)
    wg[:K * C] = W.transpose(2, 1, 0).reshape(K * C, CO)
    bias = np.ascontiguousarray(b.reshape(CO, 1))
    oblk = np.zeros((P, NB), np.float32)
    oblk[:NB * C] = -np.kron(np.eye(NB, dtype=np.float32),
                             np.ones((C, 1), np.float32))

    import ml_dtypes
    ones_plane = np.ones((NB, 3, N), np.float32)

    nc = _get_nc()
    in_maps = []
    for i in range(NCORES):
        xs = x[NB * i:NB * (i + 1)]                      # (NB, C, N) f32
        xh = _bf16(xs)
        xl = _bf16(xs - xh)
        # lhsT rows: [1,1,1, 2xh(16), 2xl(16), 2xh(16), 2xl(16)]
        lhf = np.concatenate(
            [ones_plane, 2 * xh, 2 * xl, 2 * xh, 2 * xl], axis=1)
        lhb = lhf.astype(ml_dtypes.bfloat16)
        # rhs x-rows: [xh, xh, xl, xl]
        xrf = np.concatenate([xh, xh, xl, xl], axis=1)
        xrb = xrf.astype(ml_dtypes.bfloat16)
        xrv = np.ascontiguousarray(np.concatenate([xs, xs, xs], axis=1))
        in_maps.append({"lh": np.ascontiguousarray(lhb),
                        "xr16": np.ascontiguousarray(xrb),
                        "xrep": xrv,
                        "wg": wg, "bias": bias, "oblk": oblk})
    res = run_bass_kernel_spmd(nc, in_maps, core_ids=list(range(NCORES)),
                               trace=trace)
    return np.concatenate([r["y"] for r in res.results], axis=0), res


def kernel(x: np.ndarray, W: np.ndarray, b: np.ndarray, **kw) -> np.ndarray:
    return run(x, W, b)[0]


# revision 19
# speedup vs baseline: 1.0119x; 1.0052x over previous
"""Trainium2 Bass kernel for nn_Conv1d_NN (retrieval_knn).

Reference computation per batch b (B=32, C=16, N=2048, K=3, C_out=32):
  dist[n,m] = |x[:,n] - x[:,m]|^2        (N x N pairwise distances)
  idx[n,:]  = argmin-3 of dist[n,:]      (self included)
  out[o,n]  = sum_{c,k} W[o,c,k] * x[c, idx[n,k]] + bias[o]

Kernel strategy (data-parallel over batch, 4 batches / core x 8 cores):
  * top-3 neighbours of row n are the top-3 LARGEST of
      score[n,m] = 2*gram[n,m] - sq[m]           (sq[n] row-const dropped)
    computed on-chip as ONE matmul per 128-row tile.  The matmul runs in
    bf16 hi/lo SPLIT precision (1 PE cycle/row instead of fp32's 4):
      x = xh + xl (bf16 halves), sq = sqh + sql + sqll (bf16 thirds)
      lhsT = [1;1;1; 2xh; 2xl; 2xh; 2xl]   (67 x 128 cols per tile)
      rhs  = [-sqh;-sql;-sqll; xh; xh; xl; xl]
    contraction 67 rounds up to the full-array 128 LDW path; all cross
    terms (hh, lh, hl, ll) are kept so the score error is ~1e-4 --
    top-3 selection flips only a handful of near-tied rows (rel err
    ~2-4e-3, vs 8e-4 for exact fp32, tolerance 2e-2).
  * DVE max/max_index (top-8 streaming sort) give values+indices per row.
    These two full passes (2 x 2048 cycles/tile) are the structural
    bottleneck: no TRN2 instruction recovers per-row top-k indices in
    fewer DVE cycles (Max/MaxIndex support no 2x/4x perf modes, and
    per-partition-dynamic gathers don't exist off gpsimd's 16-shared
    wrapped layout).
  * Score tile t covers rows n = 16*r + t so that a single DMA-transpose
    of the slot-major index tile lands EXACTLY in the int16 "wrapped"
    layout that gpsimd.ap_gather expects, with gather column j == n.
  * gather builds prime[(k,c), n] = x[c, idx[n,k]]; a zero-padded K=128
    matmul with W[(k,c), o] contracts gather+conv in one shot (fp32r:
    1 PE cycle/row, ~1e-4 relative noise -- output-domain only); bias is
    applied on the PSUM->SBUF move (ACT Identity with per-partition bias).

Schedule notes (DVE-bound; everything else hides under the ~302us of
Max/MaxIndex):
  * each batch's tail (index transpose / gather / conv / store) is emitted
    AFTER the next batch's score tiles to avoid PE head-of-line blocking;
  * the DMA transpose lives on the Activation DGE queue so its xbar-mode
    switch cannot serialize the SP load queue;
  * L/R are double-buffered by batch parity and prefetched one batch ahead;
    gather sources are prefetched one batch ahead on the scalar queue;
  * x^2 squaring runs on ACT (Square activation), not DVE;
  * the startup chain (x -> x^2 -> -sum x^2 -> sq splits -> R/L -> first
    score matmul) is cut into 512-column chunks so batch 0's first DVE
    Max gates on ~1/4 of the head instead of half of it.
"""

import numpy as np

import concourse.bass as bass
import concourse.bacc as bacc
import concourse.mybir as mybir
from concourse import tile
from concourse.bass_utils import run_bass_kernel_spmd

F32 = mybir.dt.float32
F32R = mybir.dt.float32r
BF16 = mybir.dt.bfloat16
U16 = mybir.dt.uint16
I16 = mybir.dt.int16
AF = mybir.ActivationFunctionType

NCORES = 8
B, C, N, K, CO = 32, 16, 2048, 3, 32
NB = B // NCORES          # batches per core
NT = N // 128             # score tiles per batch
P = 128                   # padded contraction size (conv matmul)
CK = 3 + 4 * C            # score contraction: 3 sq rows + [2xh;2xl;2xh;2xl]


def build_kernel() -> bass.Bass:
    nc = bacc.Bacc("TRN2", target_bir_lowering=False, debug=False)
    # lh[b] = [1;1;1; 2xh; 2xl; 2xh; 2xl]   (bf16 lhsT rows 0..66)
    lh = nc.dram_tensor("lh", [NB, CK, N], BF16, kind="ExternalInput")
    # xr16[b] = [xh; xh; xl; xl]            (bf16 rhs rows 3..66)
    xr16 = nc.dram_tensor("xr16", [NB, 4 * C, N], BF16, kind="ExternalInput")
    # xrep[b] = [x_b ; x_b ; x_b]           (gather source, f32)
    xrep = nc.dram_tensor("xrep", [NB, K * C, N], F32, kind="ExternalInput")
    # wg[(k*16+c), o] = W[o, c, k], zero-padded to 128 rows
    wg = nc.dram_tensor("wg", [P, CO], F32, kind="ExternalInput")
    bias = nc.dram_tensor("bias", [CO, 1], F32, kind="ExternalInput")
    # oblk = NEGATED block-diagonal ones, zero-padded to 128 rows
    oblk = nc.dram_tensor("oblk", [P, NB], F32, kind="ExternalInput")
    y = nc.dram_tensor("y", [NB, CO, N], F32, kind="ExternalOutput")

    with tile.TileContext(nc) as tc:
        with (
            tc.tile_pool(name="const", bufs=1) as cpool,
            tc.tile_pool(name="work", bufs=2) as wpool,
            tc.tile_pool(name="padded", bufs=1) as zpool,
            tc.tile_pool(name="smat", bufs=4) as spool,
            tc.tile_pool(name="small", bufs=4) as mpool,
            tc.tile_pool(name="psum", bufs=4, space="PSUM") as ppool,
        ):
            wg_sb = cpool.tile([P, CO], F32)
            nc.scalar.dma_start(wg_sb[:], wg[:])
            bias_sb = cpool.tile([CO, 1], F32)
            nc.scalar.dma_start(bias_sb[:], bias[:])
            oblk_sb = cpool.tile([P, NB], F32)
            nc.scalar.dma_start(oblk_sb[:], oblk[:])

            # xsq zero-padded to 128 rows for the oblk contraction.
            xsq = zpool.tile([P, N], F32, tag="xsq")
            nc.gpsimd.memset(xsq[:], 0.0)
            # score operands: bf16, contraction CK=67 (no padding needed --
            # 67 > 64 rounds the PE tile up to the full 128 LDW path).
            Ls, Rs = [], []
            for i in range(2):
                Ls.append(zpool.tile([CK, N], BF16, tag=f"L{i}",
                                     name=f"L{i}"))
                Rs.append(zpool.tile([CK, N], BF16, tag=f"R{i}",
                                     name=f"R{i}"))
            # conv gather target, zero rows 48..127 for the K=128 matmul
            pr = zpool.tile([P, N], F32, tag="pr")
            nc.gpsimd.memset(pr[:], 0.0)
            # gather-source tiles; loads staggered across batches (scalar DGE)
            xrs = [zpool.tile([K * C, N], F32, tag=f"xr{b}", name=f"xr{b}")
                   for b in range(NB)]
            nc.scalar.dma_start(xrs[0][:], xrep[0])

            # L and R rows 3.. are pure host data: load batch 0's (and the
            # prefetch of batch 1 handles the rest) before the sq chain so
            # the first score matmul gates only on sq3 chunk 0.
            def load_host(b):
                nc.sync.dma_start(Rs[b % 2][3:CK, :], xr16[b])
                nc.sync.dma_start(Ls[b % 2][0:CK, :], lh[b])

            load_host(0)

            # head chain in 512-column chunks: x -> x^2 (ACT) -> -sum x^2
            # (PE, exact fp32) -> sq hi/lo/lolo splits (ACT + DVE).
            x4 = cpool.tile([NB * C, N], F32)
            nsq_sb = cpool.tile([NB, N], F32)
            # sq3[b, j*N + n] = j-th bf16 third of batch b's -sq; engine
            # writes stay lane-aligned (partitions 0..3), the R-load DMA
            # unfolds the j column-blocks onto R partitions 0..2.
            sq3 = cpool.tile([NB, 3 * N], BF16)
            sq3v = sq3[:].rearrange("b (j n) -> b j n", j=3)
            r1f = cpool.tile([NB, N], F32)
            for ch in range(4):
                sl = slice(ch * 512, (ch + 1) * 512)
                nc.sync.dma_start(x4[:, sl], xrep[:, 0:C, sl])
                nc.scalar.activation(xsq[0:NB * C, sl], x4[:, sl], AF.Square)
                psq = ppool.tile([NB, 512], F32, tag="ps")
                nc.tensor.matmul(psq[:], oblk_sb[:], xsq[:, sl],
                                 start=True, stop=True)
                nc.scalar.copy(nsq_sb[:, sl], psq[:])
                # sq = three bf16 thirds (nsq is already -sq)
                # splits on DVE: its queue is empty until the first Max
                # anyway, so this rides entirely inside the startup bubble
                nc.scalar.activation(sq3v[:, 0, sl], nsq_sb[:, sl], AF.Copy)
                nc.vector.tensor_sub(r1f[:, sl], nsq_sb[:, sl],
                                     sq3v[:, 0, sl])
                nc.scalar.activation(sq3v[:, 1, sl], r1f[:, sl], AF.Copy)
                nc.vector.tensor_sub(sq3v[:, 2, sl], r1f[:, sl],
                                     sq3v[:, 1, sl])
                nc.sync.dma_start(
                    Rs[0][0:3, sl],
                    sq3[0:1, :].rearrange("b (j n) -> b j n", j=3)[:, :, sl])

            def load_LR(b):
                # R = [-sqh;-sql;-sqll; xh; xh; xl; xl], L from host.
                load_host(b)
                nc.sync.dma_start(
                    Rs[b % 2][0:3, :],
                    sq3[b:b + 1, :].rearrange("b (j n) -> b j n", j=3))

            def tail(b, idxw):
                # one DMA transpose -> ap_gather wrapped int16 layout
                # (on the Activation DGE queue: the xbar-mode switch
                #  serializes the queue, so keep it off the SP load queue)
                TT = wpool.tile([128, 128], I16, tag="TT")
                nc.scalar.dma_start(TT[:], idxw[:].bitcast(I16),
                                    transpose=True)
                # gather neighbour columns: pr[(k,c), n] = x[c, idx[n,k]]
                nc.gpsimd.ap_gather(pr[0:K * C, :], xrs[b][:], TT[0:K * C, :],
                                    channels=K * C, num_elems=N, d=1,
                                    num_idxs=N)
                # conv == contraction over (k,c); bias on the PSUM->SBUF move
                ob = wpool.tile([CO, N], F32, tag="ob")
                for h in range(2):
                    po = ppool.tile([CO, N // 2], F32, tag="ps")
                    for ch in range(2):
                        sl = slice(ch * 512, (ch + 1) * 512)
                        gl = slice(h * 1024 + ch * 512,
                                   h * 1024 + (ch + 1) * 512)
                        nc.tensor.matmul(po[:, sl], wg_sb[:], pr[:, gl],
                                         start=True, stop=True)
                    hl = slice(h * 1024, (h + 1) * 1024)
                    nc.scalar.activation(ob[:, hl], po[:], AF.Identity,
                                         bias=bias_sb[:])
                    nc.sync.dma_start(y[b, :, hl], ob[:, hl])

            pending = None
            for b in range(NB):
                R = Rs[b % 2]
                L = Ls[b % 2]

                # slot-major top-8 index tile: idxw[r, 16*slot + t]
                idxw = wpool.tile([128, 128], U16, tag="idxw")
                idxwv = idxw[:].rearrange("p (s g) -> p g s", g=16)
                Lv = L[:].rearrange("p (r g) -> p g r", g=16)

                for t in range(NT):
                    S = spool.tile([128, N], F32, tag="S")
                    for h in range(2):
                        ps = ppool.tile([128, N // 2], F32, tag="ps")
                        for ch in range(2):
                            sl = slice(ch * 512, (ch + 1) * 512)
                            gl = slice(h * 1024 + ch * 512,
                                       h * 1024 + (ch + 1) * 512)
                            nc.tensor.matmul(ps[:, sl], Lv[:, t, :], R[:, gl],
                                             start=True, stop=True)
                        nc.scalar.copy(S[:, h * 1024:(h + 1) * 1024], ps[:])
                    mx = mpool.tile([128, 8], F32, tag="mx")
                    nc.vector.max(mx[:], S[:])
                    nc.vector.max_index(idxwv[:, t, :], mx[:], S[:])
                    if t == 0 and b + 1 < NB:
                        load_LR(b + 1)   # prefetch next batch's operands
                    if t == 1 and b + 1 < NB:
                        nc.scalar.dma_start(xrs[b + 1][:], xrep[b + 1])

                # defer this batch's tail past the next batch's score tiles
                # to avoid PE head-of-line blocking on the gather chain
                if pending is not None:
                    tail(*pending)
                pending = (b, idxw)
            tail(*pending)
    nc.finalize()
    return nc


_CACHED_NC = None


def _get_nc():
    global _CACHED_NC
    if _CACHED_NC is None:
        _CACHED_NC = build_kernel()
    return _CACHED_NC


def _bf16(a: np.ndarray) -> np.ndarray:
    """Round f32 -> bf16 (round-to-nearest-even), returned as float32."""
    u = a.astype(np.float32).view(np.uint32)
    rounded = (u + 0x7FFF + ((u >> 16) & 1)) & 0xFFFF0000
    return rounded.view(np.float32)


def run(x, W, b, trace=False):
    x = np.asarray(x, dtype=np.float32)
    W = np.asarray(W, dtype=np.float32)
    b = np.asarray(b, dtype=np.float32)
    # wg[(k*16+c), o] = W[o, c, k], zero-padded to 128 rows
    wg = np.zeros((P, CO), np.float32)
    wg[:K * C] = W.transpose(2, 1, 0).reshape(K * C, CO)
    bias = np.ascontiguousarray(b.reshape(CO, 1))
    oblk = np.zeros((P, NB), np.float32)
    oblk[:NB * C] = -np.kron(np.eye(NB, dtype=np.float32),
                             np.ones((C, 1), np.float32))

    import ml_dtypes
    ones_plane = np.ones((NB, 3, N), np.float32)

    nc = _get_nc()
    in_maps = []
    for i in range(NCORES):
        xs = x[NB * i:NB * (i + 1)]                      # (NB, C, N) f32
        xh = _bf16(xs)
        xl = _bf16(xs - xh)
        # lhsT rows: [1,1,1, 2xh(16), 2xl(16), 2xh(16), 2xl(16)]
        lhf = np.concatenate(
            [ones_plane, 2 * xh, 2 * xl, 2 * xh, 2 * xl], axis=1)
        lhb = lhf.astype(ml_dtypes.bfloat16)
        # rhs x-rows: [xh, xh, xl, xl]
        xrf = np.concatenate([xh, xh, xl, xl], axis=1)
        xrb = xrf.astype(ml_dtypes.bfloat16)
        xrv = np.ascontiguousarray(np.concatenate([xs, xs, xs], axis=1))
        in_maps.append({"lh": np.ascontiguousarray(lhb),
                        "xr16": np.ascontiguousarray(xrb),
                        "xrep": xrv,
                        "wg": wg, "bias": bias, "oblk": oblk})
    res = run_bass_kernel_spmd(nc, in_maps, core_ids=list(range(NCORES)),
                               trace=trace)
    return np.concatenate([r["y"] for r in res.results], axis=0), res


def kernel(x: np.ndarray, W: np.ndarray, b: np.ndarray, **kw) -> np.ndarray:
    return run(x, W, b)[0]
